# revision 1
# baseline (speedup 1.0000x reference)
"""AttentionRetrieval kNN kernel for 8 TRN2 NeuronCores (Bass, raw Block style).

Reference math:
    qp  = query @ Wq.T + bq           (4096, 4096)   [flattened over (D=32, H=128)]
    kp  = support @ Wk.T + bk         (16384, 4096)
    sim = -(|qp|^2 + |kp|^2 - 2 qp@kp.T) / sqrt(128)
    idx, w = top16(sim), softmax(top16 values)

The per-row |qp|^2 shifts all scores of a row equally, so it affects neither
the top-k selection nor the softmax; it is dropped.  Device score:
    score = (2/sqrt(H)) * qp @ kp.T - |kp|^2 / sqrt(H)
identical ordering and weights.

Launch 1 (support sharded 8 x 2048): kpT (transposed kp), qpT (transposed qp,
pre-scaled by 2/sqrt(H)), gamma = -|kp|^2/sqrt(H) -- all on device.
Launch 2 (queries sharded 8 x 512): 512 x 16384 x 4096 fp32 matmul + gamma
add, on-chip top-16 (DVE max8 / find_index8 / match_replace) + softmax.

DMA-completion semaphores are per-ring-slot with at most one outstanding DMA
each (slot reuse is gated on consumer progress via engine semaphores), which
keeps every wait race-free under out-of-order DMA completion.
"""
import sys
sys.path.insert(0, "/opt/trn_rl_repo")
import numpy as np
import concourse.bass as bass
from concourse import mybir
from concourse.bass_utils import run_bass_kernel_spmd

f32 = mybir.dt.float32
f16 = mybir.dt.float16
u16 = mybir.dt.uint16

N_CORES = 8
NQ, NS, D, H = 4096, 16384, 32, 128
DH = D * H
NQ_SH = NQ // N_CORES           # 512
NS_SH = NS // N_CORES           # 2048
K = 16
SC = 512
SCALE_QP = 2.0 / np.sqrt(H)
SCALE_G = -1.0 / np.sqrt(H)
NEG = -1.0e30
ADD, MUL, SUB = mybir.AluOpType.add, mybir.AluOpType.mult, mybir.AluOpType.subtract


def build_launch1():
    nc = bass.Bass("TRN2", target_bir_lowering=False, debug=False, num_devices=N_CORES)
    supT = nc.dram_tensor("supT", (DH, NS_SH), f32, kind="ExternalInput")
    qT = nc.dram_tensor("qT", (DH, NQ_SH), f32, kind="ExternalInput")
    WkT = nc.dram_tensor("WkT", (H, H), f32, kind="ExternalInput")
    WqT = nc.dram_tensor("WqT", (H, H), f32, kind="ExternalInput")
    bk = nc.dram_tensor("bk", (H, 1), f32, kind="ExternalInput")
    bq = nc.dram_tensor("bq", (H, 1), f32, kind="ExternalInput")
    kpT_out = nc.dram_tensor("kpT", (DH, NS_SH), f32, kind="ExternalOutput")
    qpT_out = nc.dram_tensor("qpT", (DH, NQ_SH), f32, kind="ExternalOutput")
    g_out = nc.dram_tensor("gamma", (1, NS_SH), f32, kind="ExternalOutput")

    supT_v = supT.ap().rearrange("(g p) s -> p g s", p=H)
    qT_v = qT.ap().rearrange("(g p) n -> p g n", p=H)
    kpT_v = kpT_out.ap().rearrange("(g p) s -> p g s", p=H)
    qpT_v = qpT_out.ap().rearrange("(g p) n -> p g n", p=H)

    DG = 4
    NDG = 32 // DG              # 8 input tiles per chunk
    NCH = NS_SH // SC           # 4 s-chunks
    R_IN, R_KP, R_SQ, R_PS, R_G = 3, 4, 4, 4, 2

    sup_sb = [nc.alloc_sbuf_tensor(f"sup{i}", [H, DG, SC], f32) for i in range(R_IN)]
    kp_sb = [nc.alloc_sbuf_tensor(f"kp{i}", [H, SC], f32) for i in range(R_KP)]
    sq_sb = [nc.alloc_sbuf_tensor(f"sq{i}", [H, SC], f32) for i in range(R_SQ)]
    sqacc = [nc.alloc_sbuf_tensor(f"sqacc{i}", [H, SC], f32) for i in range(2)]
    qt_sb = [nc.alloc_sbuf_tensor(f"qt{i}", [H, DG, SC], f32) for i in range(2)]
    WkT_sb = nc.alloc_sbuf_tensor("WkT_sb", [H, H], f32)
    WqT_sb = nc.alloc_sbuf_tensor("WqT_sb", [H, H], f32)
    bk_sb = nc.alloc_sbuf_tensor("bk_sb", [H, 1], f32)
    bq_sb = nc.alloc_sbuf_tensor("bq_sb", [H, 1], f32)
    ones_sb = nc.alloc_sbuf_tensor("ones_sb", [H, 1], f32)
    g_sb = [nc.alloc_sbuf_tensor(f"g{i}", [1, SC], f32) for i in range(R_G)]

    ps_k = [nc.alloc_psum_tensor(f"psk{i}", [H, SC], f32) for i in range(R_PS)]
    ps_k2 = nc.alloc_psum_tensor("ps_ksq", [1, SC], f32)
    ps_q = [nc.alloc_psum_tensor(f"psq{i}", [H, SC], f32) for i in range(2)]

    with (
        nc.Block() as block,
        nc.semaphore("s_const") as s_const,
        nc.semaphore("s_sup0") as s_sup0,
        nc.semaphore("s_sup1") as s_sup1,
        nc.semaphore("s_sup2") as s_sup2,
        nc.semaphore("s_qt0") as s_qt0,
        nc.semaphore("s_qt1") as s_qt1,
        nc.semaphore("s_kpo0") as s_kpo0,
        nc.semaphore("s_kpo1") as s_kpo1,
        nc.semaphore("s_kpo2") as s_kpo2,
        nc.semaphore("s_kpo3") as s_kpo3,
        nc.semaphore("s_gout") as s_gout,
        nc.semaphore("pe") as pe,
        nc.semaphore("pe2") as pe2,
        nc.semaphore("dve") as dve,
        nc.semaphore("act") as act,
        nc.semaphore("gam") as gam,
        nc.semaphore("av") as av,
    ):
        s_sup = [s_sup0, s_sup1, s_sup2]
        s_qt = [s_qt0, s_qt1]
        s_kpo = [s_kpo0, s_kpo1, s_kpo2, s_kpo3]

        @block.sync
        def _(sync):
            for src, sb in ((WkT, WkT_sb), (WqT, WqT_sb), (bk, bk_sb), (bq, bq_sb)):
                sync.dma_start(out=sb[:], in_=src.ap()).then_inc(s_const, 16)
            for c in range(NCH):
                for i in range(NDG):
                    t = c * NDG + i
                    if t >= R_IN:
                        sync.wait_ge(pe, DG * (t - R_IN + 1))
                    sync.dma_start(
                        out=sup_sb[t % R_IN][:],
                        in_=supT_v[:, i * DG:(i + 1) * DG, c * SC:(c + 1) * SC],
                    ).then_inc(s_sup[t % R_IN], 16)
            for i in range(NDG):
                if i >= 2:
                    sync.wait_ge(pe, 128 + DG * (i - 1))
                sync.dma_start(
                    out=qt_sb[i % 2][:], in_=qT_v[:, i * DG:(i + 1) * DG, :]
                ).then_inc(s_qt[i % 2], 16)

        @block.tensor
        def _(tensor):
            def mm_ones_chunk(cc):
                # single column-sum matmul over the chunk's accumulated squares
                tensor.wait_ge(gam, cc)            # ps_k2 freed by gamma read cc-1
                tensor.wait_ge(av, 32 * (cc + 1))  # sqacc[cc%2] complete
                nc.tensor.matmul(
                    ps_k2[:], lhsT=ones_sb[:], rhs=sqacc[cc % 2][:],
                    start=True, stop=True,
                ).then_inc(pe2, 1)

            tensor.wait_ge(s_const, 4 * 16)
            for c in range(NCH):
                for d in range(32):
                    i, j = d // DG, d % DG
                    t = c * NDG + i
                    g = c * 32 + d
                    if d == 0 and c >= 1:
                        mm_ones_chunk(c - 1)
                    if j == 0:
                        tensor.wait_ge(s_sup[t % R_IN], 16 * (t // R_IN + 1))
                    if g >= R_PS:
                        tensor.wait_ge(dve, g - R_PS + 1)
                    nc.tensor.matmul(
                        ps_k[g % R_PS][:], lhsT=WkT_sb[:],
                        rhs=sup_sb[t % R_IN][:, j, :],
                        start=True, stop=True,
                    ).then_inc(pe, 1)
            for d in range(32):
                i, j = d // DG, d % DG
                if d == 0:
                    mm_ones_chunk(NCH - 1)
                if j == 0:
                    tensor.wait_ge(s_qt[i % 2], 16 * (i // 2 + 1))
                if d >= 2:
                    tensor.wait_ge(dve, 128 + (d - 2) + 1)
                nc.tensor.matmul(
                    ps_q[d % 2][:], lhsT=WqT_sb[:], rhs=qt_sb[i % 2][:, j, :],
                    start=True, stop=True,
                ).then_inc(pe, 1)

        @block.gpsimd
        def _(gpsimd):
            # accumulate squares across d on the otherwise-idle GPSIMD engine
            for c in range(NCH):
                for d in range(32):
                    g = c * 32 + d
                    gpsimd.wait_ge(act, g + 1)
                    if d == 0:
                        if c >= 2:
                            gpsimd.wait_ge(pe2, c - 1)   # sqacc slot reuse
                        nc.gpsimd.tensor_copy(
                            out=sqacc[c % 2][:], in_=sq_sb[g % R_SQ][:]
                        ).then_inc(av, 1)
                    else:
                        gpsimd.wait_ge(av, g)            # previous acc op done
                        nc.gpsimd.tensor_add(
                            sqacc[c % 2][:], sqacc[c % 2][:], sq_sb[g % R_SQ][:]
                        ).then_inc(av, 1)

        @block.vector
        def _(vector):
            vector.wait_ge(s_const, 4 * 16)
            nc.vector.memset(ones_sb[:], 1.0)
            for c in range(NCH):
                for d in range(32):
                    g = c * 32 + d
                    vector.wait_ge(pe, g + 1)
                    if g >= R_KP:
                        gp = g - R_KP
                        vector.wait_ge(s_kpo[g % R_KP], 16 * (gp // R_KP + 1))
                        vector.wait_ge(act, gp + 1)
                    nc.vector.tensor_scalar(
                        kp_sb[g % R_KP][:], ps_k[g % R_PS][:], bk_sb[:], None, ADD
                    ).then_inc(dve, 1)
                vector.wait_ge(pe2, c + 1)
                if c >= R_G:
                    vector.wait_ge(s_gout, 16 * (c - R_G + 1))
                nc.vector.tensor_scalar(
                    g_sb[c % R_G][:], ps_k2[:], float(SCALE_G), None, MUL
                ).then_inc(gam, 1)
            for d in range(32):
                gq = 128 + d
                gp = gq - R_KP
                vector.wait_ge(pe, gq + 1)
                vector.wait_ge(s_kpo[gq % R_KP], 16 * (gp // R_KP + 1))
                if gp < 128:
                    vector.wait_ge(act, gp + 1)
                nc.vector.tensor_scalar(
                    kp_sb[gq % R_KP][:], ps_q[d % 2][:], bq_sb[:], float(SCALE_QP),
                    ADD, op1=MUL,
                ).then_inc(dve, 1)

        @block.scalar
        def _(scalar):
            # output DMAs ride the scalar engine's separate HWDGE ring so
            # input (SP) and output (ACT) traffic split across two rings.
            for c in range(NCH):
                for d in range(32):
                    g = c * 32 + d
                    scalar.wait_ge(dve, g + 1)
                    nc.scalar.dma_start(
                        out=kpT_v[:, d, c * SC:(c + 1) * SC], in_=kp_sb[g % R_KP][:]
                    ).then_inc(s_kpo[g % R_KP], 16)
                    if g >= R_SQ:
                        scalar.wait_ge(av, g - R_SQ + 1)
                    nc.scalar.activation(
                        sq_sb[g % R_SQ][:], kp_sb[g % R_KP][:],
                        mybir.ActivationFunctionType.Square,
                    ).then_inc(act, 1)
                scalar.wait_ge(gam, c + 1)
                nc.scalar.dma_start(
                    out=g_out.ap()[:, c * SC:(c + 1) * SC], in_=g_sb[c % R_G][:]
                ).then_inc(s_gout, 16)
            for d in range(32):
                gq = 128 + d
                scalar.wait_ge(dve, gq + 1)
                nc.scalar.dma_start(
                    out=qpT_v[:, d, :], in_=kp_sb[gq % R_KP][:]
                ).then_inc(s_kpo[gq % R_KP], 16)
            # final: wait for all outstanding output DMAs
            for sl in range(R_KP):
                n_out = len([g for g in range(160) if g % R_KP == sl])
                scalar.wait_ge(s_kpo[sl], 16 * n_out)
            scalar.wait_ge(s_gout, 16 * NCH)

    return nc


def build_launch2():
    nc = bass.Bass(
        "TRN2", target_bir_lowering=False, debug=False, num_devices=N_CORES,
        dynamic_dma_scratch_size=2048,
    )
    kpT_hi = nc.dram_tensor("kpT_hi", (DH, NS), f16, kind="ExternalInput")
    kpT_lo = nc.dram_tensor("kpT_lo", (DH, NS), f16, kind="ExternalInput")
    qpT_hl = nc.dram_tensor("qpT_hl", (2, DH, NQ_SH), f16, kind="ExternalInput")
    gbc = nc.dram_tensor("gbc", (H, NS), f32, kind="ExternalInput")
    idx_out = nc.dram_tensor("idx", (4, H, K), u16, kind="ExternalOutput")
    w_out = nc.dram_tensor("w", (4, H, K), f32, kind="ExternalOutput")

    kph_v = kpT_hi.ap().rearrange("(g p) s -> p g s", p=H)
    kpl_v = kpT_lo.ap().rearrange("(g p) s -> p g s", p=H)
    qp_v = qpT_hl.ap().rearrange("t (g p) n -> p t g n", p=H)   # [128, 2, 32, 512]

    NSC = NS // SC              # 32 chunks per half
    DG = 2                      # d's per kt DMA tile
    NDG = 32 // DG              # 16 tiles per chunk
    R_KT, R_G, R_PS = 4, 2, 8

    # precomputed tk-semaphore counts: 2*NSC partial max8s per half, then per
    # block 7 chained merge ops (max, mr, max, mi, mr, mi, sub); recip/mul for
    # all blocks run at the very end (one recip per block adds 1 more each).
    subs = {}
    _c = 0
    for _half in range(2):
        _c += 2 * NSC
        for _b in range(2):
            _c += 7
            subs[_half * 2 + _b] = _c
    N_TK_MAIN = _c              # tk count after all subs (before final recips)

    kth = [nc.alloc_sbuf_tensor(f"kth{i}", [H, DG, SC], f16) for i in range(R_KT)]
    ktl = [nc.alloc_sbuf_tensor(f"ktl{i}", [H, DG, SC], f16) for i in range(R_KT)]
    g_sb = [nc.alloc_sbuf_tensor(f"gs{i}", [H, SC], f32) for i in range(R_G)]
    qpb = [nc.alloc_sbuf_tensor(f"qpb{g}", [H, 2, 32, H], f16) for g in range(4)]
    score = [nc.alloc_sbuf_tensor(f"score{b}", [H, NS], f32) for b in range(2)]
    cand = [nc.alloc_sbuf_tensor(f"cand{g}", [H, NSC * 8], f32) for g in range(4)]
    mm_sb = [nc.alloc_sbuf_tensor(f"mm{g}", [H, K], f32) for g in range(4)]
    ii_sb = [nc.alloc_sbuf_tensor(f"ii{g}", [H, K], u16) for g in range(4)]
    sm_sb = [nc.alloc_sbuf_tensor(f"sm{g}", [H, K], f32) for g in range(4)]
    ex_sb = [nc.alloc_sbuf_tensor(f"ex{g}", [H, K], f32) for g in range(4)]
    w_sb = [nc.alloc_sbuf_tensor(f"wv{g}", [H, K], f32) for g in range(4)]
    sum_sb = [nc.alloc_sbuf_tensor(f"su{g}", [H, 1], f32) for g in range(4)]
    rs_sb = [nc.alloc_sbuf_tensor(f"rrs{g}", [H, 1], f32) for g in range(4)]

    ps = [nc.alloc_psum_tensor(f"ps{i}", [H, SC], f32) for i in range(R_PS)]

    tkc = {"v": 0}   # python-side tk counter

    from contextlib import ExitStack
    with ExitStack() as stack:
        block = stack.enter_context(nc.Block())
        sem = lambda name: stack.enter_context(nc.semaphore(name))
        s_qp = [sem(f"s_qp{i}") for i in range(4)]
        s_kh = [sem(f"s_kh{i}") for i in range(4)]
        s_kl = [sem(f"s_kl{i}") for i in range(4)]
        s_g = [sem(f"s_g{i}") for i in range(2)]
        s_out = sem("s_out")
        pe = sem("pe")
        pet = sem("pet")
        dve = sem("dve")
        tk = sem("tk")
        act = sem("act")
        rdy = sem("rdy")

        @block.sync
        def _(sync):
            # hi-tiles + qp on the SP HWDGE ring
            for g in range(4):
                sync.dma_start(
                    out=qpb[g][:], in_=qp_v[:, :, :, g * H:(g + 1) * H]
                ).then_inc(s_qp[g], 16)
            for half in range(2):
                for sc in range(NSC):
                    S = half * NSC + sc
                    for i in range(NDG):
                        t = S * NDG + i
                        if t >= R_KT:
                            sync.wait_ge(pet, t - R_KT + 1)
                        sync.dma_start(
                            out=kth[t % R_KT][:],
                            in_=kph_v[:, i * DG:(i + 1) * DG, sc * SC:(sc + 1) * SC],
                        ).then_inc(s_kh[t % R_KT], 16)
            n_out = 0
            for gblk in range(4):
                sync.wait_ge(rdy, gblk + 1)
                sync.dma_start(out=idx_out.ap()[gblk], in_=ii_sb[gblk][:]).then_inc(s_out, 16)
                sync.dma_start(out=w_out.ap()[gblk], in_=w_sb[gblk][:]).then_inc(s_out, 16)
                n_out += 2
            sync.wait_ge(s_out, 16 * n_out)

        @block.scalar
        def _(scalar):
            # lo-tiles + gamma on the ACT HWDGE ring; also the softmax exps
            for half in range(2):
                for sc in range(NSC):
                    S = half * NSC + sc
                    for i in range(NDG):
                        t = S * NDG + i
                        if t >= R_KT:
                            scalar.wait_ge(pet, t - R_KT + 1)
                        nc.scalar.dma_start(
                            out=ktl[t % R_KT][:],
                            in_=kpl_v[:, i * DG:(i + 1) * DG, sc * SC:(sc + 1) * SC],
                        ).then_inc(s_kl[t % R_KT], 16)
                    if S >= R_G:
                        scalar.wait_ge(dve, 2 * (S - R_G + 1))
                    nc.scalar.dma_start(
                        out=g_sb[S % R_G][:], in_=gbc.ap()[:, sc * SC:(sc + 1) * SC]
                    ).then_inc(s_g[S % R_G], 16)
            # softmax exps at the end so they never block half-1 prefetch
            for gblk in range(4):
                scalar.wait_ge(tk, subs[gblk])
                nc.scalar.activation(
                    ex_sb[gblk][:], sm_sb[gblk][:],
                    mybir.ActivationFunctionType.Exp,
                    accum_out=sum_sb[gblk][:],
                ).then_inc(act, 1)

        @block.tensor
        def _(tensor):
            for half in range(2):
                for b in range(2):
                    tensor.wait_ge(s_qp[half * 2 + b], 16)
                for sc in range(NSC):
                    S = half * NSC + sc
                    for d in range(32):
                        i, j = d // DG, d % DG
                        t = S * NDG + i
                        if j == 0:
                            tensor.wait_ge(s_kh[t % R_KT], 16 * (t // R_KT + 1))
                            tensor.wait_ge(s_kl[t % R_KT], 16 * (t // R_KT + 1))
                        for b in range(2):
                            g = 2 * S + b
                            if d == 0 and g >= R_PS:
                                tensor.wait_ge(dve, g - R_PS + 1)
                            qh = qpb[half * 2 + b][:, 0, d, :]
                            ql = qpb[half * 2 + b][:, 1, d, :]
                            for pno, (lhs, rhs) in enumerate((
                                (qh, kth[t % R_KT][:, j, :]),
                                (qh, ktl[t % R_KT][:, j, :]),
                                (ql, kth[t % R_KT][:, j, :]),
                            )):
                                inst = nc.tensor.matmul(
                                    ps[g % R_PS][:], lhsT=lhs, rhs=rhs,
                                    start=(d == 0 and pno == 0),
                                    stop=(d == 31 and pno == 2),
                                )
                            if d == 31 and b == 0:
                                inst.then_inc(pe, 1)   # pe = #completed b0 groups
                            if j == DG - 1 and b == 1:
                                inst.then_inc(pet, 1)  # pet = #fully-consumed kt tiles

        @block.vector
        def _(vector):
            def tkop(inst):
                inst.then_inc(tk, 1)
                tkc["v"] += 1
                return tkc["v"]

            for half in range(2):
                for sc in range(NSC):
                    S = half * NSC + sc
                    if half == 1 and sc == 0:
                        vector.wait_ge(tk, tkc["v"])   # half-0 topk reads of score done
                    for b in range(2):
                        g = 2 * S + b
                        gblk = half * 2 + b
                        if b == 0:
                            vector.wait_ge(pe, S + 1)
                            vector.wait_ge(s_g[S % R_G], 16 * (S // R_G + 1))
                        else:
                            vector.wait_ge(pet, NDG * (S + 1))
                        nc.vector.tensor_tensor(
                            out=score[b][:, sc * SC:(sc + 1) * SC],
                            in0=ps[g % R_PS][:], in1=g_sb[S % R_G][:], op=ADD,
                        ).then_inc(dve, 1)
                        # per-chunk top-8 candidates (same-engine RAW: wait the add)
                        vector.wait_ge(dve, 2 * S + b + 1)
                        tkop(nc.vector.max(
                            out=cand[gblk][:, sc * 8:sc * 8 + 8],
                            in_=score[b][:, sc * SC:(sc + 1) * SC],
                        ))
                # merge + index extraction + softmax per block
                for b in range(2):
                    gblk = half * 2 + b
                    s = score[b]
                    vector.wait_ge(tk, tkc["v"])   # partial max8s of this half done
                    c = tkop(nc.vector.max(out=mm_sb[gblk][:, 0:8], in_=cand[gblk][:]))
                    vector.wait_ge(tk, c)
                    c = tkop(nc.vector.match_replace(
                        out=cand[gblk][:], in_to_replace=mm_sb[gblk][:, 0:8],
                        in_values=cand[gblk][:], imm_value=NEG,
                    ))
                    vector.wait_ge(tk, c)
                    c = tkop(nc.vector.max(out=mm_sb[gblk][:, 8:16], in_=cand[gblk][:]))
                    vector.wait_ge(tk, c)
                    c = tkop(nc.vector.max_index(
                        out=ii_sb[gblk][:, 0:8], in_max=mm_sb[gblk][:, 0:8], in_values=s[:]
                    ))
                    vector.wait_ge(tk, c)
                    c = tkop(nc.vector.match_replace(
                        out=s[:], in_to_replace=mm_sb[gblk][:, 0:8], in_values=s[:],
                        imm_value=NEG,
                    ))
                    vector.wait_ge(tk, c)
                    c = tkop(nc.vector.max_index(
                        out=ii_sb[gblk][:, 8:16], in_max=mm_sb[gblk][:, 8:16], in_values=s[:]
                    ))
                    vector.wait_ge(tk, c)
                    c = tkop(nc.vector.tensor_scalar(
                        sm_sb[gblk][:], mm_sb[gblk][:], mm_sb[gblk][:, 0:1], None, SUB
                    ))
                    assert c == subs[gblk], (c, gblk, subs)
            # softmax normalization for all blocks at the very end (keeps the
            # ACT exp handshake out of the inter-half critical path)
            for gblk in range(4):
                vector.wait_ge(act, gblk + 1)
                c = tkop(nc.vector.reciprocal(rs_sb[gblk][:], sum_sb[gblk][:]))
                vector.wait_ge(tk, c)
                nc.vector.tensor_scalar(
                    w_sb[gblk][:], ex_sb[gblk][:], rs_sb[gblk][:], None, MUL
                ).then_inc(rdy, 1)

    return nc


_CACHE = {}


def _get_programs():
    if "l1" not in _CACHE:
        _CACHE["l1"] = build_launch1()
        _CACHE["l2"] = build_launch2()
    return _CACHE["l1"], _CACHE["l2"]


def run_launches(query, support, Wq, bq, Wk, bk, trace2=False, trace1=False):
    nc1, nc2 = _get_programs()

    supT = np.ascontiguousarray(support.reshape(NS, DH).T)
    qT = np.ascontiguousarray(query.reshape(NQ, DH).T)
    WkT_a = np.ascontiguousarray(Wk.T)
    WqT_a = np.ascontiguousarray(Wq.T)
    bk_c = np.ascontiguousarray(bk.reshape(H, 1))
    bq_c = np.ascontiguousarray(bq.reshape(H, 1))

    in_maps1 = [
        {
            "supT": np.ascontiguousarray(supT[:, c * NS_SH:(c + 1) * NS_SH]),
            "qT": np.ascontiguousarray(qT[:, c * NQ_SH:(c + 1) * NQ_SH]),
            "WkT": WkT_a, "WqT": WqT_a, "bk": bk_c, "bq": bq_c,
        }
        for c in range(N_CORES)
    ]
    res1 = run_bass_kernel_spmd(
        nc1, in_maps1, core_ids=list(range(N_CORES)), trace=trace1
    )

    kpT_full = np.concatenate([res1.results[c]["kpT"] for c in range(N_CORES)], axis=1)
    gamma = np.concatenate([res1.results[c]["gamma"][0] for c in range(N_CORES)])
    gbc_a = np.ascontiguousarray(np.broadcast_to(gamma, (H, NS)))

    # exact fp16 hi/lo decomposition: x == hi + lo to ~2^-22 relative
    kp_hi16 = kpT_full.astype(np.float16)
    kp_lo16 = (kpT_full - kp_hi16.astype(np.float32)).astype(np.float16)

    in_maps2 = []
    for c in range(N_CORES):
        qp = res1.results[c]["qpT"]
        qp_hi = qp.astype(np.float16)
        qp_lo = (qp - qp_hi.astype(np.float32)).astype(np.float16)
        in_maps2.append({
            "kpT_hi": kp_hi16, "kpT_lo": kp_lo16, "gbc": gbc_a,
            "qpT_hl": np.ascontiguousarray(np.stack([qp_hi, qp_lo])),
        })
    res2 = run_bass_kernel_spmd(
        nc2, in_maps2, core_ids=list(range(N_CORES)), trace=trace2
    )

    idx = np.concatenate(
        [res2.results[c]["idx"].reshape(NQ_SH, K) for c in range(N_CORES)], axis=0
    ).astype(np.int32)
    w = np.concatenate(
        [res2.results[c]["w"].reshape(NQ_SH, K) for c in range(N_CORES)], axis=0
    )
    return idx, w, (res1, res2)


def kernel(query, support, Wq, bq, Wk, bk, k):
    assert int(k) == K
    query = np.asarray(query, np.float32)
    support = np.asarray(support, np.float32)
    Wq = np.asarray(Wq, np.float32)
    bq = np.asarray(bq, np.float32)
    Wk = np.asarray(Wk, np.float32)
    bk = np.asarray(bk, np.float32)
    idx, w, _ = run_launches(query, support, Wq, bq, Wk, bk)
    return idx, w



# revision 6
# speedup vs baseline: 12.2595x; 12.2595x over previous
"""AttentionRetrieval kNN kernel for 8 TRN2 NeuronCores (Bass, raw Block style).

Reference math:
    qp  = query @ Wq.T + bq           (4096, 4096)   [flattened over (D=32, H=128)]
    kp  = support @ Wk.T + bk         (16384, 4096)
    sim = -(|qp|^2 + |kp|^2 - 2 qp@kp.T) / sqrt(128)
    idx, w = top16(sim), softmax(top16 values)

Fused formulation (per-row constants drop out of topk and softmax):
    score[i,j] = sum_d (q_d M) s_d^T [i,j] + g[j]
      M  = (2/sqrt(H)) Wq^T Wk                  (queries projected once, host)
      g  = -|s Wk^T + (bk - bq)|^2 / sqrt(H)    (completed square folds the
                                                 bq-cross-term; global consts drop)
so launch 2 streams the RAW transposed support once — no kpT materialization.

Launch 1 (support sharded 8 x 2048): fp32 (exact) projection with bias
(bk - bq), square + column-sum -> g shard (1 x 2048, 8 KB out per core).

Launch 2 (queries sharded 8 x 512): single-pass float32r matmul
(qm_d stationary, raw supT moving; f32r = RNE-11-bit input rounding at
1 cycle/row — 3x fewer PE rows than an exact hi/lo scheme), + g add, and
per-512-chunk top-8 (DVE max8 + max_index) -> 256 candidates/row.

Host: merge 256 candidates -> top-24, flag rows whose top-17 adjacent gaps
are below the f32r noise bound, exactly rescore flagged rows in f64
(24 dot products each), then top-16 + softmax. Flip rate vs the fp32
reference matches an exact device kernel (~2 rows from fp32 tie noise).
"""
import sys
sys.path.insert(0, "/opt/trn_rl_repo")
import numpy as np
import concourse.bass as bass
from concourse import mybir
from concourse.bass_utils import run_bass_kernel_spmd

f32 = mybir.dt.float32
f32r = mybir.dt.float32r
u16 = mybir.dt.uint16

N_CORES = 8
NQ, NS, D, H = 4096, 16384, 32, 128
DH = D * H
NQ_SH = NQ // N_CORES           # 512
NS_SH = NS // N_CORES           # 2048
K = 16
SC = 512
MCAND = 24                      # host merge keeps top-24 candidates per row
TAU = 0.026                     # rescore-flag threshold (~8 sigma of f32r noise)
SCALE_G = -1.0 / np.sqrt(H)
ADD, MUL = mybir.AluOpType.add, mybir.AluOpType.mult


def build_launch1():
    """Per-core: g = -|supT_shard.T @ Wk.T + (bk-bq)|^2 / sqrt(H), fp32-exact."""
    nc = bass.Bass("TRN2", target_bir_lowering=False, debug=False, num_devices=N_CORES)
    supT = nc.dram_tensor("supT", (DH, NS_SH), f32, kind="ExternalInput")
    WkT = nc.dram_tensor("WkT", (H, H), f32, kind="ExternalInput")
    bp = nc.dram_tensor("bp", (H, 1), f32, kind="ExternalInput")
    g_out = nc.dram_tensor("g", (1, NS_SH), f32, kind="ExternalOutput")

    supT_v = supT.ap().rearrange("(g p) s -> p g s", p=H)

    DG = 4
    NDG = 32 // DG              # 8 input tiles per chunk
    NCH1 = NS_SH // SC          # 4 s-chunks
    R_IN, R_KP, R_SQ, R_PS, R_G = 3, 4, 4, 4, 2

    sup_sb = [nc.alloc_sbuf_tensor(f"sup{i}", [H, DG, SC], f32) for i in range(R_IN)]
    kp_sb = [nc.alloc_sbuf_tensor(f"kp{i}", [H, SC], f32) for i in range(R_KP)]
    sq_sb = [nc.alloc_sbuf_tensor(f"sq{i}", [H, SC], f32) for i in range(R_SQ)]
    sqacc = [nc.alloc_sbuf_tensor(f"sqacc{i}", [H, SC], f32) for i in range(2)]
    WkT_sb = nc.alloc_sbuf_tensor("WkT_sb", [H, H], f32)
    bp_sb = nc.alloc_sbuf_tensor("bp_sb", [H, 1], f32)
    ones_sb = nc.alloc_sbuf_tensor("ones_sb", [H, 1], f32)
    g_sb = [nc.alloc_sbuf_tensor(f"g{i}", [1, SC], f32) for i in range(R_G)]

    ps_k = [nc.alloc_psum_tensor(f"psk{i}", [H, SC], f32) for i in range(R_PS)]
    ps_k2 = nc.alloc_psum_tensor("ps_ksq", [1, SC], f32)

    with (
        nc.Block() as block,
        nc.semaphore("s_const") as s_const,
        nc.semaphore("s_sup0") as s_sup0,
        nc.semaphore("s_sup1") as s_sup1,
        nc.semaphore("s_sup2") as s_sup2,
        nc.semaphore("s_gout") as s_gout,
        nc.semaphore("pe") as pe,
        nc.semaphore("pe2") as pe2,
        nc.semaphore("dve") as dve,
        nc.semaphore("act") as act,
        nc.semaphore("gam") as gam,
        nc.semaphore("av") as av,
    ):
        s_sup = [s_sup0, s_sup1, s_sup2]

        @block.sync
        def _(sync):
            for src, sb in ((WkT, WkT_sb), (bp, bp_sb)):
                sync.dma_start(out=sb[:], in_=src.ap()).then_inc(s_const, 16)
            for c in range(NCH1):
                for i in range(NDG):
                    t = c * NDG + i
                    if t >= R_IN:
                        sync.wait_ge(pe, DG * (t - R_IN + 1))
                    sync.dma_start(
                        out=sup_sb[t % R_IN][:],
                        in_=supT_v[:, i * DG:(i + 1) * DG, c * SC:(c + 1) * SC],
                    ).then_inc(s_sup[t % R_IN], 16)

        @block.tensor
        def _(tensor):
            def mm_ones_chunk(cc):
                tensor.wait_ge(gam, cc)            # ps_k2 freed by gamma read cc-1
                tensor.wait_ge(av, 32 * (cc + 1))  # sqacc[cc%2] complete
                nc.tensor.matmul(
                    ps_k2[:], lhsT=ones_sb[:], rhs=sqacc[cc % 2][:],
                    start=True, stop=True,
                ).then_inc(pe2, 1)

            tensor.wait_ge(s_const, 2 * 16)
            for c in range(NCH1):
                for d in range(32):
                    i, j = d // DG, d % DG
                    t = c * NDG + i
                    g = c * 32 + d
                    if d == 0 and c >= 1:
                        mm_ones_chunk(c - 1)
                    if j == 0:
                        tensor.wait_ge(s_sup[t % R_IN], 16 * (t // R_IN + 1))
                    if g >= R_PS:
                        tensor.wait_ge(dve, g - R_PS + 1)
                    nc.tensor.matmul(
                        ps_k[g % R_PS][:], lhsT=WkT_sb[:],
                        rhs=sup_sb[t % R_IN][:, j, :],
                        start=True, stop=True,
                    ).then_inc(pe, 1)
            mm_ones_chunk(NCH1 - 1)

        @block.gpsimd
        def _(gpsimd):
            for c in range(NCH1):
                for d in range(32):
                    g = c * 32 + d
                    gpsimd.wait_ge(act, g + 1)
                    if d == 0:
                        if c >= 2:
                            gpsimd.wait_ge(pe2, c - 1)   # sqacc slot reuse
                        nc.gpsimd.tensor_copy(
                            out=sqacc[c % 2][:], in_=sq_sb[g % R_SQ][:]
                        ).then_inc(av, 1)
                    else:
                        gpsimd.wait_ge(av, g)            # previous acc op done
                        nc.gpsimd.tensor_add(
                            sqacc[c % 2][:], sqacc[c % 2][:], sq_sb[g % R_SQ][:]
                        ).then_inc(av, 1)

        @block.vector
        def _(vector):
            vector.wait_ge(s_const, 2 * 16)
            nc.vector.memset(ones_sb[:], 1.0)
            for c in range(NCH1):
                for d in range(32):
                    g = c * 32 + d
                    vector.wait_ge(pe, g + 1)
                    if g >= R_KP:
                        vector.wait_ge(act, g - R_KP + 1)   # kp_sb slot reuse
                    nc.vector.tensor_scalar(
                        kp_sb[g % R_KP][:], ps_k[g % R_PS][:], bp_sb[:], None, ADD
                    ).then_inc(dve, 1)
                vector.wait_ge(pe2, c + 1)
                if c >= R_G:
                    vector.wait_ge(s_gout, 16 * (c - R_G + 1))
                nc.vector.tensor_scalar(
                    g_sb[c % R_G][:], ps_k2[:], float(SCALE_G), None, MUL
                ).then_inc(gam, 1)

        @block.scalar
        def _(scalar):
            for c in range(NCH1):
                for d in range(32):
                    g = c * 32 + d
                    scalar.wait_ge(dve, g + 1)
                    if g >= R_SQ:
                        scalar.wait_ge(av, g - R_SQ + 1)
                    nc.scalar.activation(
                        sq_sb[g % R_SQ][:], kp_sb[g % R_KP][:],
                        mybir.ActivationFunctionType.Square,
                    ).then_inc(act, 1)
                scalar.wait_ge(gam, c + 1)
                nc.scalar.dma_start(
                    out=g_out.ap()[:, c * SC:(c + 1) * SC], in_=g_sb[c % R_G][:]
                ).then_inc(s_gout, 16)
            scalar.wait_ge(s_gout, 16 * NCH1)

    return nc


def build_launch2():
    """Per-core: 512 queries x 16384 supports, 1-pass f32r + per-chunk top-8."""
    nc = bass.Bass("TRN2", target_bir_lowering=False, debug=False, num_devices=N_CORES)
    supT = nc.dram_tensor("supT", (DH, NS), f32r, kind="ExternalInput")
    qmT = nc.dram_tensor("qmT", (DH, NQ_SH), f32r, kind="ExternalInput")
    gbc = nc.dram_tensor("gbc", (H, NS), f32, kind="ExternalInput")
    cval_out = nc.dram_tensor("cval", (4, H, 256), f32, kind="ExternalOutput")
    cidx_out = nc.dram_tensor("cidx", (4, H, 256), u16, kind="ExternalOutput")

    sup_v = supT.ap().rearrange("(g p) s -> p g s", p=H)    # [128, 32, 16384]
    qm_v = qmT.ap().rearrange("(g p) n -> p g n", p=H)      # [128, 32, 512]

    NCH2 = NS // SC             # 32 chunks
    DG = 4
    NDG = 32 // DG              # 8 sup tiles per chunk
    NT = NCH2 * NDG             # 256 sup tiles
    R_KT, R_G, R_SCB = 10, 4, 6

    qm_sb = nc.alloc_sbuf_tensor("qm_sb", [H, 32, NQ_SH], f32r)
    kt_sb = [nc.alloc_sbuf_tensor(f"kt{i}", [H, DG, SC], f32r) for i in range(R_KT)]
    g_sb = [nc.alloc_sbuf_tensor(f"gs{i}", [H, SC], f32) for i in range(R_G)]
    sc_sb = [nc.alloc_sbuf_tensor(f"scb{i}", [H, SC], f32) for i in range(R_SCB)]
    cv_sb = [nc.alloc_sbuf_tensor(f"cv{b}", [H, 256], f32) for b in range(4)]
    ci_sb = [nc.alloc_sbuf_tensor(f"ci{b}", [H, 256], u16) for b in range(4)]

    ps = [nc.alloc_psum_tensor(f"ps{i}", [H, SC], f32) for i in range(8)]

    with (
        nc.Block() as block,
        nc.semaphore("s_qm") as s_qm,
        nc.semaphore("s_kt0") as s_kt0,
        nc.semaphore("s_kt1") as s_kt1,
        nc.semaphore("s_kt2") as s_kt2,
        nc.semaphore("s_kt3") as s_kt3,
        nc.semaphore("s_kt4") as s_kt4,
        nc.semaphore("s_kt5") as s_kt5,
        nc.semaphore("s_kt6") as s_kt6,
        nc.semaphore("s_kt7") as s_kt7,
        nc.semaphore("s_kt8") as s_kt8,
        nc.semaphore("s_kt9") as s_kt9,
        nc.semaphore("s_g0") as s_g0,
        nc.semaphore("s_g1") as s_g1,
        nc.semaphore("s_g2") as s_g2,
        nc.semaphore("s_g3") as s_g3,
        nc.semaphore("s_out") as s_out,
        nc.semaphore("pe") as pe,
        nc.semaphore("pet") as pet,
        nc.semaphore("dve") as dve,
        nc.semaphore("tk") as tk,
    ):
        s_kt = [s_kt0, s_kt1, s_kt2, s_kt3, s_kt4, s_kt5, s_kt6, s_kt7, s_kt8, s_kt9]
        s_g = [s_g0, s_g1, s_g2, s_g3]

        @block.sync
        def _(sync):
            # qm pieces interleaved ahead of the first sup tiles
            for i in range(NDG):
                sync.dma_start(
                    out=qm_sb[:, i * DG:(i + 1) * DG, :],
                    in_=qm_v[:, i * DG:(i + 1) * DG, :],
                ).then_inc(s_qm, 16)
                t = i          # first chunk's tiles
                sync.dma_start(
                    out=kt_sb[t % R_KT][:],
                    in_=sup_v[:, t % NDG * DG:(t % NDG + 1) * DG, 0:SC],
                ).then_inc(s_kt[t % R_KT], 16)
            for t in range(NDG, NT):
                c, i = t // NDG, t % NDG
                if t >= R_KT:
                    sync.wait_ge(pet, t - R_KT + 1)
                sync.dma_start(
                    out=kt_sb[t % R_KT][:],
                    in_=sup_v[:, i * DG:(i + 1) * DG, c * SC:(c + 1) * SC],
                ).then_inc(s_kt[t % R_KT], 16)

        @block.scalar
        def _(scalar):
            # g tiles on the ACT HWDGE ring + final candidate output DMAs
            for c in range(NCH2):
                if c >= R_G:
                    scalar.wait_ge(dve, 4 * (c - R_G) + 4)   # slot's adds done
                nc.scalar.dma_start(
                    out=g_sb[c % R_G][:], in_=gbc.ap()[:, c * SC:(c + 1) * SC]
                ).then_inc(s_g[c % R_G], 16)
            scalar.wait_ge(tk, 8 * NCH2)
            for b in range(4):
                nc.scalar.dma_start(out=cval_out.ap()[b], in_=cv_sb[b][:]).then_inc(s_out, 16)
                nc.scalar.dma_start(out=cidx_out.ap()[b], in_=ci_sb[b][:]).then_inc(s_out, 16)
            scalar.wait_ge(s_out, 16 * 8)

        @block.tensor
        def _(tensor):
            for c in range(NCH2):
                for d in range(32):
                    i, j = d // DG, d % DG
                    t = c * NDG + i
                    if c == 0 and j == 0:
                        tensor.wait_ge(s_qm, 16 * (i + 1))
                    if j == 0:
                        tensor.wait_ge(s_kt[t % R_KT], 16 * (t // R_KT + 1))
                    for b in range(4):
                        cell = 4 * c + b
                        if d == 0 and cell >= 8:
                            tensor.wait_ge(dve, cell - 8 + 1)   # psum bank freed
                        inst = nc.tensor.matmul(
                            ps[(c % 2) * 4 + b][:],
                            lhsT=qm_sb[:, d, b * H:(b + 1) * H],
                            rhs=kt_sb[t % R_KT][:, j, :],
                            start=(d == 0), stop=(d == 31),
                        )
                        # one semaphore update per instruction: b0..b2 stops
                        # mark pe (3/chunk); the b3/d31 stop marks its tile's
                        # pet (which also implies the whole chunk finished).
                        if d == 31 and b < 3:
                            inst.then_inc(pe, 1)
                        elif j == DG - 1 and b == 3:
                            inst.then_inc(pet, 1)   # sup tile fully consumed

        @block.vector
        def _(vector):
            for c in range(NCH2):
                vector.wait_ge(s_g[c % R_G], 16 * (c // R_G + 1))
                for b in range(4):
                    cell = 4 * c + b
                    if b < 3:
                        vector.wait_ge(pe, 3 * c + b + 1)
                    else:
                        vector.wait_ge(pet, NDG * (c + 1))
                    nc.vector.tensor_tensor(
                        out=sc_sb[cell % R_SCB][:],
                        in0=ps[(c % 2) * 4 + b][:], in1=g_sb[c % R_G][:], op=ADD,
                    ).then_inc(dve, 1)
                    vector.wait_ge(dve, cell + 1)       # same-engine RAW
                    nc.vector.max(
                        out=cv_sb[b][:, c * 8:c * 8 + 8],
                        in_=sc_sb[cell % R_SCB][:],
                    ).then_inc(tk, 1)
                    vector.wait_ge(tk, 2 * cell + 1)    # same-engine RAW
                    nc.vector.max_index(
                        out=ci_sb[b][:, c * 8:c * 8 + 8],
                        in_max=cv_sb[b][:, c * 8:c * 8 + 8],
                        in_values=sc_sb[cell % R_SCB][:],
                    ).then_inc(tk, 1)

    return nc


_CACHE = {}


def _get_programs():
    if "l1" not in _CACHE:
        _CACHE["l1"] = build_launch1()
        _CACHE["l2"] = build_launch2()
    return _CACHE["l1"], _CACHE["l2"]


def run_launches(query, support, Wq, bq, Wk, bk, trace2=False, trace1=False):
    nc1, nc2 = _get_programs()

    sflat = np.ascontiguousarray(support.reshape(NS, DH))
    supT = np.ascontiguousarray(sflat.T)
    WkT_a = np.ascontiguousarray(Wk.T)
    bp = np.ascontiguousarray((bk - bq).reshape(H, 1))

    in_maps1 = [
        {
            "supT": np.ascontiguousarray(supT[:, c * NS_SH:(c + 1) * NS_SH]),
            "WkT": WkT_a, "bp": bp,
        }
        for c in range(N_CORES)
    ]
    res1 = run_bass_kernel_spmd(
        nc1, in_maps1, core_ids=list(range(N_CORES)), trace=trace1
    )
    gvec = np.concatenate([res1.results[c]["g"][0] for c in range(N_CORES)])

    M = ((Wq.T @ Wk) * np.float32(2.0 / np.sqrt(H))).astype(np.float32)
    qm = (query.reshape(NQ * D, H) @ M).reshape(NQ, DH)
    gbc_a = np.ascontiguousarray(np.broadcast_to(gvec, (H, NS)))

    in_maps2 = [
        {
            "supT": supT,
            "qmT": np.ascontiguousarray(qm[c * NQ_SH:(c + 1) * NQ_SH].T),
            "gbc": gbc_a,
        }
        for c in range(N_CORES)
    ]
    res2 = run_bass_kernel_spmd(
        nc2, in_maps2, core_ids=list(range(N_CORES)), trace=trace2
    )

    # ---- host merge: (4, H, 256) per core -> (NQ, 256) candidate vals/idx
    NCH2 = NS // SC
    cvals = np.empty((NQ, NCH2 * 8), np.float32)
    cidx = np.empty((NQ, NCH2 * 8), np.int64)
    base = (np.arange(NCH2, dtype=np.int64) * SC).repeat(8)[None, :]
    for c in range(N_CORES):
        cv = res2.results[c]["cval"].reshape(4 * H, NCH2 * 8)
        ci = res2.results[c]["cidx"].reshape(4 * H, NCH2 * 8).astype(np.int64)
        cvals[c * NQ_SH:(c + 1) * NQ_SH] = cv
        cidx[c * NQ_SH:(c + 1) * NQ_SH] = ci + base

    part = np.argpartition(-cvals, MCAND, 1)[:, :MCAND]
    pv = np.take_along_axis(cvals, part, 1)
    pi = np.take_along_axis(cidx, part, 1)
    order = np.lexsort((pi, -pv), axis=1)
    pv = np.take_along_axis(pv, order, 1)
    pi = np.take_along_axis(pi, order, 1)

    # flag rows whose top-17 adjacent gaps could be reordered by f32r noise
    flag = ((pv[:, :16] - pv[:, 1:17]) < TAU).any(1)
    fr = np.where(flag)[0]
    if fr.size:
        sel = sflat[pi[fr].ravel()].reshape(fr.size, MCAND, DH)
        ex = np.einsum(
            "nd,ncd->nc", qm[fr], sel, dtype=np.float64, optimize=True
        ) + gvec[pi[fr]]
        o2 = np.lexsort((pi[fr], -ex), axis=1)
        pv[fr] = np.take_along_axis(ex, o2, 1).astype(np.float32)
        pi[fr] = np.take_along_axis(pi[fr], o2, 1)

    idx = pi[:, :16].astype(np.int32)
    tv = pv[:, :16].astype(np.float64)
    e = np.exp(tv - tv[:, :1])
    w = (e / e.sum(1, keepdims=True)).astype(np.float32)
    return idx, w, (res1, res2)


def kernel(query, support, Wq, bq, Wk, bk, k):
    assert int(k) == K
    query = np.asarray(query, np.float32)
    support = np.asarray(support, np.float32)
    Wq = np.asarray(Wq, np.float32)
    bq = np.asarray(bq, np.float32)
    Wk = np.asarray(Wk, np.float32)
    bk = np.asarray(bk, np.float32)
    idx, w, _ = run_launches(query, support, Wq, bq, Wk, bk)
    return idx, w
